# revision 10
# baseline (speedup 1.0000x reference)
"""Trainium2 Bass kernel for GQA attention (B=1, S=2048, D=2048, H=16, KVH=4, HD=128).

Strategy (sequence parallel over query blocks, 8 cores, ZERO collectives):
  - Core c owns query blocks {c, 15-c} (128 rows each, 256 q rows) — the
    balanced causal split. Every core redundantly computes the FULL K/V
    (SPMD-uniform program); causality is enforced purely through per-core
    additive mask DATA, so all cores run the identical instruction stream.
  - No ReduceScatter/AllReduce at all: each core's 256 output rows are
    disjoint; the host just concatenates. (Collectives accounted for ~55 ms
    of the previous head-sharded kernel's ~55 ms body time.)
  - bf16 everywhere on the matmul path (PSUM accumulation stays fp32):
    1 cycle/row on the PE (4x over fp32), half the DMA bytes and SBUF.
    Host pre-casts inputs; on-chip casts fold into existing copies.
  - Head-dim permutation trick: wq/wk columns are permuted per head to
    [even dims, odd dims] so RoPE becomes two contiguous 64-partition
    halves. q/k permuted identically => scores unchanged; v untouched.
  - K computed transposed (kT [hd, s]); V computed in natural [s, hd]
    layout directly (stationary = xT tiles), no PE transposes.
  - Attention fully transposed per head: scoresT [k, q] quads, one mask
    add + one exp per quad, PV with v natural, denominators via a
    ones-matmul, normalization via one batched DRAM-bounce broadcast.
  - Output projection in natural layout (stationary = attnT tiles),
    streamed wo; result rows DMA straight to the output tensor.
  - Weights/activations are host-packed so every weight DMA is a single
    fully-contiguous [128, N] transfer.
"""

import numpy as np
import ml_dtypes
from contextlib import ExitStack

import concourse.bacc as bacc
import concourse.tile as tile
import concourse.mybir as mybir
from concourse.bass_utils import run_bass_kernel_spmd

S = 2048
D = 2048
H = 16
KVH = 4
HD = 128
NCORES = 8
NQ = 256          # q rows per core (2 blocks of 128)
F32 = mybir.dt.float32
BF16 = mybir.dt.bfloat16
NPBF16 = ml_dtypes.bfloat16
SCALE = float(1.0 / np.sqrt(HD))
NEG = -1e9

_BUILD_CACHE = {}


def _emit_body(nc, tc, io):
    mm = nc.tensor.matmul
    with ExitStack() as ctx:
        # ---------------- whole-body constants ----------------
        consts = ctx.enter_context(tc.tile_pool(name="consts", bufs=1))
        ones_sb = consts.tile([128, 1], BF16, tag="ones")
        nc.vector.memset(ones_sb[:], 1.0)

        # attnT survives into the output-projection phase
        attp = ctx.enter_context(tc.tile_pool(name="attp", bufs=1))
        attnT_sb = attp.tile([128, 16, 256], BF16, tag="attnT")  # [hd, h, q]

        with tc.tile_pool(name="acts", bufs=1) as acts:
            kT_sb = acts.tile([128, 4, 2048], BF16, tag="kT")    # [hd, g, s]
            v_sb = acts.tile([128, 4, 16, 128], BF16, tag="v")   # [k, g, kt, hd]
            qT_sb = acts.tile([128, 16, 256], BF16, tag="qT")    # [hd, h, q]

            # ---------------- phase 1: K (transposed) + V (natural) ----------
            with tc.tile_pool(name="wkv_pool", bufs=1) as wkvp, \
                 tc.tile_pool(name="xp", bufs=2) as xpool, \
                 tc.tile_pool(name="pproj", bufs=1, space="PSUM") as pproj:
                wkv_sb = wkvp.tile([128, 16, 1024], BF16, tag="wkv")
                nc.sync.dma_start(
                    out=wkv_sb[:].rearrange("p t n -> p (t n)"),
                    in_=io["wkv"][:])
                for c2 in range(2):  # s super-chunks of 1024
                    x_sc = xpool.tile([128, 16, 1024], BF16, tag="xsc")
                    for dt in range(16):
                        nc.sync.dma_start(
                            out=x_sc[:, dt, :],
                            in_=io["xT"][dt * 128:(dt + 1) * 128,
                                         c2 * 1024:(c2 + 1) * 1024])
                    for sub in range(2):  # s chunks of 512
                        ch = 2 * c2 + sub
                        ps_k = pproj.tile([128, 4, 512], F32, tag="psk")
                        ps_v = pproj.tile([128, 4, 512], F32, tag="psv")
                        for dt in range(16):
                            xm = x_sc[:, dt, sub * 512:(sub + 1) * 512]
                            st = dt == 0
                            sp = dt == 15
                            for g in range(4):
                                mm(ps_k[:, g, :],
                                   wkv_sb[:, dt, g * 128:(g + 1) * 128],
                                   xm, start=st, stop=sp)
                            for sl in range(4):
                                mm(ps_v[:, sl, :],
                                   xm[:, sl * 128:(sl + 1) * 128],
                                   wkv_sb[:, dt, 512:1024], start=st, stop=sp)
                        nc.vector.tensor_copy(
                            kT_sb[:, :, ch * 512:(ch + 1) * 512], ps_k[:])
                        # ps_v[:, sl, :] holds v rows [ch*512+sl*128 .. +128),
                        # cols [4 kv-heads x 128] -> v_sb [k, g, ktile, hd]
                        for sl in range(4):
                            nc.vector.tensor_copy(
                                v_sb[:, :, ch * 4 + sl, :],
                                ps_v[:, sl, :].rearrange("p (g n) -> p g n",
                                                         g=4))

            # ---------------- phase 1b: Q projection (transposed) ------------
            with tc.tile_pool(name="wq_pool", bufs=3) as wqp, \
                 tc.tile_pool(name="xqp", bufs=1) as xqp, \
                 tc.tile_pool(name="pq", bufs=2, space="PSUM") as pq:
                xq_sb = xqp.tile([128, 16, 256], BF16, tag="xq")
                nc.sync.dma_start(
                    out=xq_sb[:].rearrange("p t n -> p (t n)"),
                    in_=io["xqT"][:])
                for hp in range(8):  # head pairs
                    wq_sb = wqp.tile([128, 16, 256], BF16, tag="wq")
                    nc.sync.dma_start(
                        out=wq_sb[:].rearrange("p t n -> p (t n)"),
                        in_=io["wq"][hp * 128:(hp + 1) * 128, :])
                    ps_q = pq.tile([128, 2, 512], F32, tag="psq")
                    for dt in range(16):
                        st = dt == 0
                        sp = dt == 15
                        for i in range(2):
                            mm(ps_q[:, i, 0:256],
                               wq_sb[:, dt, i * 128:(i + 1) * 128],
                               xq_sb[:, dt, :], start=st, stop=sp)
                    nc.vector.tensor_copy(qT_sb[:, 2 * hp:2 * hp + 2, :],
                                          ps_q[:, :, 0:256])

            # ---------------- phase 1.5: RoPE on k and q (in place) ----------
            # DVE ops are lane-locked: bring odd halves (partitions 64-127)
            # down, rotate on partitions 0-63, DMA o1 back up.
            with tc.tile_pool(name="rope", bufs=1) as rp:
                cosk_sb = rp.tile([64, 2048], BF16, tag="cosk")
                nc.sync.dma_start(out=cosk_sb[:], in_=io["cosK"][:])
                sink_sb = rp.tile([64, 2048], BF16, tag="sink")
                nc.sync.dma_start(out=sink_sb[:], in_=io["sinK"][:])
                coskb = cosk_sb[:].unsqueeze(1).broadcast_to((64, 2, 2048))
                sinkb = sink_sb[:].unsqueeze(1).broadcast_to((64, 2, 2048))
                for gp in range(2):  # kv-head pairs, to bound SBUF usage
                    ksl = kT_sb[:, 2 * gp:2 * gp + 2, :]
                    t0 = ksl[0:64, :, :]
                    t1c = rp.tile([64, 2, 2048], BF16, tag="t1c")
                    nc.sync.dma_start(out=t1c[:], in_=ksl[64:128, :, :])
                    o1 = rp.tile([64, 2, 2048], BF16, tag="o1")
                    tmp = rp.tile([64, 2, 2048], BF16, tag="tmp")
                    nc.vector.tensor_mul(o1[:], t0, sinkb)
                    nc.vector.tensor_mul(tmp[:], t1c[:], coskb)
                    nc.vector.tensor_add(o1[:], o1[:], tmp[:])
                    nc.vector.tensor_mul(tmp[:], t1c[:], sinkb)
                    nc.vector.tensor_mul(t1c[:], t0, coskb)
                    nc.vector.tensor_sub(ksl[0:64, :, :], t1c[:], tmp[:])
                    nc.sync.dma_start(out=ksl[64:128, :, :], in_=o1[:])

                cosq_sb = rp.tile([64, 256], BF16, tag="cosq")
                nc.sync.dma_start(out=cosq_sb[:], in_=io["cosQ"][:])
                sinq_sb = rp.tile([64, 256], BF16, tag="sinq")
                nc.sync.dma_start(out=sinq_sb[:], in_=io["sinQ"][:])
                cosqb = cosq_sb[:].unsqueeze(1).broadcast_to((64, 16, 256))
                sinqb = sinq_sb[:].unsqueeze(1).broadcast_to((64, 16, 256))
                q0 = qT_sb[0:64, :, :]
                q1c = rp.tile([64, 16, 256], BF16, tag="t1c")
                nc.sync.dma_start(out=q1c[:], in_=qT_sb[64:128, :, :])
                qo1 = rp.tile([64, 16, 256], BF16, tag="o1")
                qtmp = rp.tile([64, 16, 256], BF16, tag="tmp")
                nc.vector.tensor_mul(qo1[:], q0, sinqb)
                nc.vector.tensor_mul(qtmp[:], q1c[:], cosqb)
                nc.vector.tensor_add(qo1[:], qo1[:], qtmp[:])
                nc.vector.tensor_mul(qtmp[:], q1c[:], sinqb)
                nc.vector.tensor_mul(q1c[:], q0, cosqb)
                nc.vector.tensor_sub(qT_sb[0:64, :, :], q1c[:], qtmp[:])
                nc.sync.dma_start(out=qT_sb[64:128, :, :], in_=qo1[:])

            # ---------------- phase 2: attention (per head, transposed) ------
            with tc.tile_pool(name="ph3", bufs=1) as ph3, \
                 tc.tile_pool(name="dram", bufs=1, space="DRAM") as dram, \
                 tc.tile_pool(name="psc", bufs=1, space="PSUM") as psc, \
                 tc.tile_pool(name="ppv", bufs=2, space="PSUM") as ppv, \
                 tc.tile_pool(name="pden", bufs=2, space="PSUM") as pden:
                maskT_sb = ph3.tile([128, 16, 256], F32, tag="maskT")
                nc.sync.dma_start(
                    out=maskT_sb[:].rearrange("p t n -> p (t n)"),
                    in_=io["maskT"][:])
                probs_sb = ph3.tile([128, 16, 256], BF16, tag="probs")
                recip_sb = ph3.tile([1, 4096], F32, tag="recip")
                for h in range(H):
                    g = h // 4
                    for grp in range(4):
                        ps_s = psc.tile([128, 4, 512], F32, tag="ps_s")
                        for i in range(4):
                            j = 4 * grp + i
                            mm(ps_s[:, i, 0:256],
                               kT_sb[:, g, j * 128:(j + 1) * 128],
                               qT_sb[:, h, :], start=True, stop=True)
                        nc.vector.tensor_add(
                            ps_s[:, :, 0:256], ps_s[:, :, 0:256],
                            maskT_sb[:, 4 * grp:4 * grp + 4, :])
                        nc.scalar.activation(
                            probs_sb[:, 4 * grp:4 * grp + 4, :],
                            ps_s[:, :, 0:256],
                            mybir.ActivationFunctionType.Exp, scale=SCALE)
                    ps_o = ppv.tile([128, 256], F32, tag="ps_o")
                    ps_den = pden.tile([1, 256], F32, tag="ps_den")
                    for j in range(16):
                        mm(ps_o[:], v_sb[:, g, j, :], probs_sb[:, j, :],
                           start=j == 0, stop=j == 15)
                        mm(ps_den[:], ones_sb[:], probs_sb[:, j, :],
                           start=j == 0, stop=j == 15)
                    nc.vector.tensor_copy(attnT_sb[:, h, :], ps_o[:])
                    nc.vector.reciprocal(
                        recip_sb[:, h * 256:(h + 1) * 256], ps_den[:])

                # normalize all heads at once via a DRAM-bounce broadcast
                rb = dram.tile([1, 4096], F32, name="rb")
                nc.sync.dma_start(out=rb[:], in_=recip_sb[:])
                bc = ph3.tile([128, 16, 256], F32, tag="bc")
                nc.sync.dma_start(out=bc[:].rearrange("p h q -> p (h q)"),
                                  in_=rb.to_broadcast((128, 4096)))
                nc.vector.tensor_mul(attnT_sb[:], attnT_sb[:], bc[:])

        # ---------------- phase 3: output projection (natural layout) --------
        with tc.tile_pool(name="wo_pool", bufs=2) as wop, \
             tc.tile_pool(name="osb", bufs=2) as osbp, \
             tc.tile_pool(name="pwo", bufs=2, space="PSUM") as pwo:
            for n in range(4):
                wo_sb = wop.tile([128, 16, 512], BF16, tag="wo")
                nc.sync.dma_start(
                    out=wo_sb[:].rearrange("p t n -> p (t n)"),
                    in_=io["wo"][n * 128:(n + 1) * 128, :])
                o_sb = osbp.tile([128, 2, 512], F32, tag="osb")
                for qt in range(2):
                    ps_wo = pwo.tile([128, 512], F32, tag="ps_wo")
                    for h in range(16):
                        mm(ps_wo[:],
                           attnT_sb[:, h, qt * 128:(qt + 1) * 128],
                           wo_sb[:, h, :], start=h == 0, stop=h == 15)
                    nc.vector.tensor_copy(o_sb[:, qt, :], ps_wo[:])
                nc.sync.dma_start(
                    out=io["out"][:, n * 512:(n + 1) * 512].rearrange(
                        "(a p) n -> p a n", p=128),
                    in_=o_sb[:])


def build(repeat=1, num_devices=NCORES):
    key = (repeat, num_devices)
    if key in _BUILD_CACHE:
        return _BUILD_CACHE[key]
    nc = bacc.Bacc("TRN2", target_bir_lowering=False, debug=False,
                   num_devices=num_devices)
    io = {
        "xT": nc.dram_tensor("xT", [D, S], BF16, kind="ExternalInput").ap(),
        "xqT": nc.dram_tensor("xqT", [128, 4096], BF16,
                              kind="ExternalInput").ap(),
        "wkv": nc.dram_tensor("wkv", [128, 16384], BF16,
                              kind="ExternalInput").ap(),
        "wq": nc.dram_tensor("wq", [1024, 4096], BF16,
                             kind="ExternalInput").ap(),
        "wo": nc.dram_tensor("wo", [512, 8192], BF16,
                             kind="ExternalInput").ap(),
        "cosK": nc.dram_tensor("cosK", [64, S], BF16,
                               kind="ExternalInput").ap(),
        "sinK": nc.dram_tensor("sinK", [64, S], BF16,
                               kind="ExternalInput").ap(),
        "cosQ": nc.dram_tensor("cosQ", [64, NQ], BF16,
                               kind="ExternalInput").ap(),
        "sinQ": nc.dram_tensor("sinQ", [64, NQ], BF16,
                               kind="ExternalInput").ap(),
        "maskT": nc.dram_tensor("maskT", [128, 4096], F32,
                                kind="ExternalInput").ap(),
        "out": nc.dram_tensor("out", [NQ, D], F32, kind="ExternalOutput").ap(),
    }
    with tile.TileContext(nc) as tc:
        for _ in range(repeat):
            _emit_body(nc, tc, io)
    nc.compile()
    _BUILD_CACHE[key] = nc
    return nc


def prepare_in_maps(x, wq, wk, wv, wo, freqs_cos, freqs_sin):
    bf = lambda a: np.ascontiguousarray(a).astype(NPBF16)
    x2d = np.asarray(x, dtype=np.float32).reshape(S, D)
    xT = bf(x2d.T)
    cosT = np.ascontiguousarray(np.asarray(freqs_cos, np.float32).T)  # [64, S]
    sinT = np.ascontiguousarray(np.asarray(freqs_sin, np.float32).T)

    # even dims first, then odd dims (applied to q and k only)
    perm = np.concatenate([np.arange(0, HD, 2), np.arange(1, HD, 2)])

    wq = np.asarray(wq, np.float32)
    wk = np.asarray(wk, np.float32)
    wv = np.asarray(wv, np.float32)
    wo = np.asarray(wo, np.float32)

    # permuted wq grouped by head pairs, packed [8*128, 16*256]:
    # row hp*128+p, col dt*256 + i*128 + hd  =  wqP[dt*128+p, head 2hp+i, hd]
    wqP = wq.reshape(D, H, HD)[:, :, perm]                  # [D, 16, 128]
    wqH = bf(wqP.reshape(16, 128, 8, 2 * HD)                # [dt, p, hp, 256]
             .transpose(2, 1, 0, 3).reshape(8 * 128, 16 * 256))
    # permuted wk + wv combined, packed [128, 16*1024]
    wkP = wk.reshape(D, KVH, HD)[:, :, perm].reshape(D, KVH * HD)
    wkv = np.concatenate([wkP, wv], axis=1)                 # [D, 1024]
    wkvH = bf(wkv.reshape(16, 128, 1024).transpose(1, 0, 2).reshape(128, 16384))
    # wo by n-chunks, packed [4*128, 16*512]:
    # row n*128+p, col dt*512 + m  =  wo[dt*128+p, n*512+m]
    woH = bf(wo.reshape(16, 128, 4, 512).transpose(2, 1, 0, 3)
             .reshape(4 * 128, 16 * 512))

    kl = np.arange(128)
    in_maps = []
    for c in range(NCORES):
        qblocks = (c, 15 - c)
        qrows = np.concatenate([np.arange(b * 128, (b + 1) * 128)
                                for b in qblocks])
        # xq packed [128, 16*256]: col dt*256 + n = x[qrows[n], dt*128+p]
        xqH = bf(x2d[qrows].reshape(256, 16, 128).transpose(2, 1, 0)
                 .reshape(128, 4096))
        cosQ = bf(cosT[:, qrows])
        sinQ = bf(sinT[:, qrows])
        # maskT[kl, j, qcol]: visible iff key 128*j+kl <= qpos(qcol)
        qpos = qrows[None, None, :]                          # [1, 1, 256]
        kpos = kl[:, None, None] + 128 * np.arange(16)[None, :, None]
        maskT = np.where(kpos <= qpos, 0.0, NEG).astype(np.float32)
        in_maps.append({
            "xT": xT,
            "xqT": xqH,
            "wkv": wkvH,
            "wq": wqH,
            "wo": woH,
            "cosK": bf(cosT),
            "sinK": bf(sinT),
            "cosQ": cosQ,
            "sinQ": sinQ,
            "maskT": np.ascontiguousarray(maskT.reshape(128, 4096)),
        })
    return in_maps


def assemble_output(results):
    full = np.empty((S, D), np.float32)
    for c in range(NCORES):
        o = results[c]["out"]
        full[c * 128:(c + 1) * 128] = o[0:128]
        full[(15 - c) * 128:(16 - c) * 128] = o[128:256]
    return full.reshape(1, S, D)


def kernel(x, wq, wk, wv, wo, freqs_cos, freqs_sin, mask):
    nc = build()
    in_maps = prepare_in_maps(x, wq, wk, wv, wo, freqs_cos, freqs_sin)
    res = run_bass_kernel_spmd(nc, in_maps, core_ids=list(range(NCORES)))
    return assemble_output(res.results).astype(np.float32)


# revision 13
# speedup vs baseline: 7.6740x; 7.6740x over previous
"""Trainium2 Bass kernel for GQA attention (B=1, S=2048, D=2048, H=16, KVH=4, HD=128).

Strategy, driven by this environment's measured cost model (per *unique*
instruction ~40-90us, per loop-executed instruction ~5-15us, DMA ~50us,
collective ~1.2ms floor; actual FLOPs/bytes nearly free):

  - Tensor parallel over heads (core c: q-heads {2c, 2c+1}, kv-head c//2),
    which minimizes per-core matmul work; one bf16 ReduceScatter of the
    transposed output at the end (host reassembles + transposes).
  - Nearly the whole kernel lives inside nested hardware loops (tc.For_i)
    with STATIC SBUF/PSUM addresses: moving operands use register offsets
    (bass.ds), and matmul stationary operands - which walrus cannot
    register-offset - are staged into fixed tiles with DVE copies whose
    *source* is register-offset. This shrinks the instruction stream from
    ~2000 to ~150 instructions.
  - PSUM accumulation across loop iterations uses memset + start=False.
  - bf16 everywhere on the matmul path (fp32 PSUM), host pre-casts/packs
    so each input is a single contiguous DMA.
  - RoPE head-dim permutation trick: wq/wk columns permuted per head to
    [even|odd] so RoPE is two contiguous 64-partition halves.
  - Causality via a full additive mask tensor indexed by (ktile, qchunk)
    inside the loops (identical on all cores).
"""

import numpy as np
import ml_dtypes
from contextlib import ExitStack

import concourse.bacc as bacc
import concourse.bass as bass
import concourse.tile as tile
import concourse.mybir as mybir
from concourse.bass_utils import run_bass_kernel_spmd

S = 2048
D = 2048
H = 16
KVH = 4
HD = 128
NCORES = 8
F32 = mybir.dt.float32
BF16 = mybir.dt.bfloat16
NPBF16 = ml_dtypes.bfloat16
SCALE = float(1.0 / np.sqrt(HD))
NEG = -1e9

_BUILD_CACHE = {}


def _emit_body(nc, tc, io):
    mm = nc.tensor.matmul
    ds = bass.ds
    with ExitStack() as ctx:
        sb = ctx.enter_context(tc.tile_pool(name="sb", bufs=1))
        dram = ctx.enter_context(tc.tile_pool(name="dram", bufs=1, space="DRAM"))

        ones_sb = sb.tile([128, 1], BF16, tag="ones")
        nc.vector.memset(ones_sb[:], 1.0)
        ident_sb = sb.tile([128, 128], BF16, tag="ident")
        nc.sync.dma_start(out=ident_sb[:], in_=io["ident"][:])
        wqkv_sb = sb.tile([128, 8192], BF16, tag="wqkv")
        nc.sync.dma_start(out=wqkv_sb[:], in_=io["wqkv"][:])
        wo_sb = sb.tile([128, 4096], BF16, tag="wo")
        nc.sync.dma_start(out=wo_sb[:], in_=io["wo2"][:])
        cossin_sb = sb.tile([64, 4096], BF16, tag="cossin")
        nc.sync.dma_start(out=cossin_sb[:], in_=io["cossin"][:])
        mask_sb = sb.tile([128, 32768], BF16, tag="mask")
        nc.sync.dma_start(out=mask_sb[:], in_=io["maskF"][:])

        # persistent activations: qk = [q0 | q1 | kT] each [128, 2048]
        qk_sb = sb.tile([128, 3, 2048], BF16, tag="qk")
        vT_sb = sb.tile([128, 2048], BF16, tag="vT")
        v_sb = sb.tile([128, 2048], BF16, tag="v")
        attnT_sb = sb.tile([128, 4096], BF16, tag="attnT")   # [hd, (h, q)]
        recip_sb = sb.tile([1, 4096], F32, tag="recip")

        xT_r = io["xT"].rearrange("(t p) s -> p t s", p=128)  # [128, 16, 2048]

        # ---------------- phase 1: QKV projections (transposed) --------------
        # loop sc (s-chunks of 512): stream x chunk, stage wqkv[dt], 4 MMs/dt
        with tc.tile_pool(name="p1", bufs=1) as p1, \
             tc.tile_pool(name="p1s", bufs=1) as p1s, \
             tc.tile_pool(name="pp1", bufs=1, space="PSUM") as pp1:
            ps_p = pp1.tile([128, 4, 512], F32, tag="ps_p")  # q0,q1,k,v
            xch = p1.tile([128, 16, 512], BF16, tag="xch")
            wst = p1s.tile([128, 512], BF16, tag="wst")

            def p1_body(sc, unroll=1):
                nc.sync.dma_start(out=xch[:], in_=xT_r[:, :, ds(sc * 512, 512)])
                nc.vector.memset(ps_p[:], 0.0)
                for dt in range(16):
                    nc.vector.tensor_copy(wst[:], wqkv_sb[:, ds(dt * 512, 512)])
                    for j in range(4):
                        mm(ps_p[:, j, :], wst[:, j * 128:(j + 1) * 128],
                           xch[:, dt, :], start=False, stop=False)
                for j in range(3):
                    nc.vector.tensor_copy(qk_sb[:, j, ds(sc * 512, 512)],
                                          ps_p[:, j, :])
                nc.vector.tensor_copy(vT_sb[:, ds(sc * 512, 512)], ps_p[:, 3, :])

            with tc.For_i(0, 4, 1) as sc:
                p1_body(sc)

        # ---------------- phase 1b: vT -> v natural (PE transposes) ----------
        with tc.tile_pool(name="p2s", bufs=1) as p2s, \
             tc.tile_pool(name="pp2", bufs=1, space="PSUM") as pp2:
            tst = p2s.tile([128, 128], BF16, tag="tst")
            ps_t = pp2.tile([128, 128], BF16, tag="ps_t")

            def p2_body(kt, unroll=1):
                nc.vector.tensor_copy(tst[:], vT_sb[:, ds(kt * 128, 128)])
                nc.tensor.transpose(ps_t[:], tst[:], ident_sb[:])
                nc.vector.tensor_copy(v_sb[:, ds(kt * 128, 128)], ps_t[:])

            with tc.For_i(0, 16, 1) as kt:
                p2_body(kt)

        # ---------------- phase 1.5: RoPE on q0, q1, k (in place) ------------
        with tc.tile_pool(name="rp", bufs=1) as rp:
            cosb = cossin_sb[:, 0:2048].unsqueeze(1).broadcast_to((64, 3, 2048))
            sinb = cossin_sb[:, 2048:4096].unsqueeze(1).broadcast_to((64, 3, 2048))
            t0 = qk_sb[0:64, :, :]
            t1c = rp.tile([64, 3, 2048], BF16, tag="t1c")
            nc.sync.dma_start(out=t1c[:], in_=qk_sb[64:128, :, :])
            o1 = rp.tile([64, 3, 2048], BF16, tag="o1")
            tmp = rp.tile([64, 3, 2048], BF16, tag="tmp")
            nc.vector.tensor_mul(o1[:], t0, sinb)
            nc.vector.tensor_mul(tmp[:], t1c[:], cosb)
            nc.vector.tensor_add(o1[:], o1[:], tmp[:])
            nc.vector.tensor_mul(tmp[:], t1c[:], sinb)
            nc.vector.tensor_mul(t1c[:], t0, cosb)
            nc.vector.tensor_sub(qk_sb[0:64, :, :], t1c[:], tmp[:])
            nc.sync.dma_start(out=qk_sb[64:128, :, :], in_=o1[:])

        # ---------------- phase 2: attention (h x qc x kt loops) -------------
        qk_f = qk_sb[:].rearrange("p a b -> p (a b)")        # [128, 6144]
        with tc.tile_pool(name="p3s", bufs=1) as p3s, \
             tc.tile_pool(name="pp3", bufs=1, space="PSUM") as pp3:
            kst = p3s.tile([128, 128], BF16, tag="kst")
            vst = p3s.tile([128, 128], BF16, tag="vst")
            probs = p3s.tile([128, 512], BF16, tag="probs")
            ps_sc = pp3.tile([128, 512], F32, tag="ps_sc")
            ps_pv = pp3.tile([128, 512], F32, tag="ps_pv")
            ps_den = pp3.tile([1, 512], F32, tag="ps_den")

            def kt_body(h, qc, kt, unroll=1):
                nc.vector.tensor_copy(kst[:], qk_f[:, ds(4096 + kt * 128, 128)])
                nc.vector.tensor_copy(vst[:], v_sb[:, ds(kt * 128, 128)])
                mm(ps_sc[:], kst[:], qk_f[:, ds(h * 2048 + qc * 512, 512)],
                   start=True, stop=True)
                nc.vector.tensor_add(
                    ps_sc[:], ps_sc[:],
                    mask_sb[:, ds(kt * 2048 + qc * 512, 512)])
                nc.scalar.activation(probs[:], ps_sc[:],
                                     mybir.ActivationFunctionType.Exp,
                                     scale=SCALE)
                mm(ps_pv[:], vst[:], probs[:], start=False, stop=False)
                mm(ps_den[:], ones_sb[:], probs[:], start=False, stop=False)

            def qc_body(h, qc, unroll=1):
                nc.vector.memset(ps_pv[:], 0.0)
                nc.vector.memset(ps_den[:], 0.0)
                with tc.For_i(0, 16, 1) as kt:
                    kt_body(h, qc, kt)
                nc.vector.tensor_copy(attnT_sb[:, ds(h * 2048 + qc * 512, 512)],
                                      ps_pv[:])
                nc.vector.reciprocal(recip_sb[:, ds(h * 2048 + qc * 512, 512)],
                                     ps_den[:])

            with tc.For_i(0, 2, 1) as h:
                with tc.For_i(0, 4, 1) as qc:
                    qc_body(h, qc)

        # normalize: DRAM-bounce broadcast of 1/den, one big multiply
        rb = dram.tile([1, 4096], F32, name="rb")
        nc.sync.dma_start(out=rb[:], in_=recip_sb[:])
        bc = sb.tile([128, 4096], F32, tag="bc")
        nc.sync.dma_start(out=bc[:], in_=rb.to_broadcast((128, 4096)))
        nc.vector.tensor_mul(attnT_sb[:], attnT_sb[:], bc[:])

        # ---------------- phase 3: output projection (outT = wo^T attnT) -----
        # outT[n, q] = sum_f wo[f, n] attnT[f, q]; loop over (nh, qc)
        woaccT = dram.tile([128, 16, 2048], BF16, name="woaccT")
        with tc.tile_pool(name="p4s", bufs=1) as p4s, \
             tc.tile_pool(name="pp4", bufs=1, space="PSUM") as pp4:
            wst4 = p4s.tile([128, 2, 1024], BF16, tag="wst4")
            o_sb = p4s.tile([128, 8, 512], BF16, tag="osb")
            ps_wo = pp4.tile([128, 8, 512], F32, tag="ps_wo")

            def p4_body(qc, unroll=1):
                for nh in range(2):
                    # wo packed as [p, nh, f, n1024]
                    nc.vector.tensor_copy(
                        wst4[:].rearrange("p a b -> p (a b)"),
                        wo_sb[:, nh * 2048:(nh + 1) * 2048])
                    for nt in range(8):
                        for f in range(2):
                            mm(ps_wo[:, nt, :],
                               wst4[:, f, nt * 128:(nt + 1) * 128],
                               attnT_sb[:, ds(f * 2048 + qc * 512, 512)],
                               start=f == 0, stop=f == 1)
                    nc.vector.tensor_copy(o_sb[:], ps_wo[:])
                    nc.sync.dma_start(
                        out=woaccT[:, nh * 8:(nh + 1) * 8, ds(qc * 512, 512)],
                        in_=o_sb[:])

            with tc.For_i(0, 4, 1) as qc:
                p4_body(qc)

        # ---------------- ReduceScatter over cores (bf16) --------------------
        rs_out = dram.tile([16, 16, 2048], BF16, name="rsout")
        nc.gpsimd.collective_compute(
            "ReduceScatter", mybir.AluOpType.add,
            replica_groups=[list(range(NCORES))],
            ins=[woaccT.opt()], outs=[rs_out.opt()])
        nc.sync.dma_start(out=io["out"][:], in_=rs_out[:])


def build(repeat=1, num_devices=NCORES):
    key = (repeat, num_devices)
    if key in _BUILD_CACHE:
        return _BUILD_CACHE[key]
    nc = bacc.Bacc("TRN2", target_bir_lowering=False, debug=False,
                   num_devices=num_devices)
    io = {
        "xT": nc.dram_tensor("xT", [D, S], BF16, kind="ExternalInput").ap(),
        "wqkv": nc.dram_tensor("wqkv", [128, 8192], BF16,
                               kind="ExternalInput").ap(),
        "wo2": nc.dram_tensor("wo2", [128, 4096], BF16,
                              kind="ExternalInput").ap(),
        "cossin": nc.dram_tensor("cossin", [64, 4096], BF16,
                                 kind="ExternalInput").ap(),
        "maskF": nc.dram_tensor("maskF", [128, 32768], BF16,
                                kind="ExternalInput").ap(),
        "ident": nc.dram_tensor("ident", [128, 128], BF16,
                                kind="ExternalInput").ap(),
        "out": nc.dram_tensor("out", [16, 16, 2048], BF16,
                              kind="ExternalOutput").ap(),
    }
    with tile.TileContext(nc) as tc:
        for _ in range(repeat):
            _emit_body(nc, tc, io)
    nc.compile()
    _BUILD_CACHE[key] = nc
    return nc


def prepare_in_maps(x, wq, wk, wv, wo, freqs_cos, freqs_sin):
    bf = lambda a: np.ascontiguousarray(a).astype(NPBF16)
    x2d = np.asarray(x, dtype=np.float32).reshape(S, D)
    xT = bf(x2d.T)
    cosT = np.asarray(freqs_cos, np.float32).T                # [64, S]
    sinT = np.asarray(freqs_sin, np.float32).T
    cossin = bf(np.concatenate([cosT, sinT], axis=1))         # [64, 4096]

    perm = np.concatenate([np.arange(0, HD, 2), np.arange(1, HD, 2)])
    wq = np.asarray(wq, np.float32)
    wk = np.asarray(wk, np.float32)
    wv = np.asarray(wv, np.float32)
    wo = np.asarray(wo, np.float32)
    wqP = wq.reshape(D, H, HD)[:, :, perm]                    # [D, 16, 128]
    wkP = wk.reshape(D, KVH, HD)[:, :, perm]                  # [D, 4, 128]
    wv4 = wv.reshape(D, KVH, HD)

    # full causal mask in scoresT layout: [kl, kt, qc, ql]
    kl = np.arange(128)[:, None, None, None]
    ktv = np.arange(16)[None, :, None, None]
    qcv = np.arange(4)[None, None, :, None]
    qlv = np.arange(512)[None, None, None, :]
    maskF = np.where(128 * ktv + kl <= 512 * qcv + qlv, 0.0, NEG)
    maskF = bf(maskF.reshape(128, 32768))

    ident = bf(np.eye(128, dtype=np.float32))

    in_maps = []
    for c in range(NCORES):
        g = c // 2
        # wqkv packed [p, dt*512 + (q0|q1|k|v)*128 + col]
        wqkv = np.stack([wqP[:, 2 * c, :], wqP[:, 2 * c + 1, :],
                         wkP[:, g, :], wv4[:, g, :]], axis=1)  # [D, 4, 128]
        wqkv = bf(wqkv.reshape(16, 128, 512).transpose(1, 0, 2)
                  .reshape(128, 8192))
        # wo rows for this core's heads, packed [p, nh, f, n1024]
        woc = wo[256 * c:256 * c + 256, :]                    # [256, 2048]
        wo2 = bf(woc.reshape(2, 128, 2, 1024).transpose(1, 2, 0, 3)
                 .reshape(128, 4096))
        in_maps.append({
            "xT": xT,
            "wqkv": wqkv,
            "wo2": wo2,
            "cossin": cossin,
            "maskF": maskF,
            "ident": ident,
        })
    return in_maps


def assemble_output(results):
    # outT[nt*128 + 16*c + pp, q] = results[c]["out"][pp, nt, q]
    outT = np.empty((2048, 2048), np.float32)
    for c in range(NCORES):
        o = np.asarray(results[c]["out"], np.float32)         # [16, 16, 2048]
        for nt in range(16):
            outT[nt * 128 + 16 * c: nt * 128 + 16 * c + 16, :] = o[:, nt, :]
    return np.ascontiguousarray(outT.T).reshape(1, S, D)


def kernel(x, wq, wk, wv, wo, freqs_cos, freqs_sin, mask):
    nc = build()
    in_maps = prepare_in_maps(x, wq, wk, wv, wo, freqs_cos, freqs_sin)
    res = run_bass_kernel_spmd(nc, in_maps, core_ids=list(range(NCORES)))
    return assemble_output(res.results).astype(np.float32)


# revision 15
# speedup vs baseline: 10.3286x; 1.3459x over previous
"""Trainium2 Bass kernel for GQA attention (B=1, S=2048, D=2048, H=16, KVH=4, HD=128).

Strategy, driven by this environment's measured cost model (per *unique*
instruction ~40-90us, per loop-executed instruction ~5-15us, DMA ~50us,
collective ~1.2ms floor; actual FLOPs/bytes nearly free):

  - Tensor parallel over heads (core c: q-heads {2c, 2c+1}, kv-head c//2),
    which minimizes per-core matmul work; one bf16 ReduceScatter of the
    transposed output at the end (host reassembles + transposes).
  - Nearly the whole kernel lives inside nested hardware loops (tc.For_i)
    with STATIC SBUF/PSUM addresses: moving operands use register offsets
    (bass.ds), and matmul stationary operands - which walrus cannot
    register-offset - are staged into fixed tiles with DVE copies whose
    *source* is register-offset. This shrinks the instruction stream from
    ~2000 to ~150 instructions.
  - PSUM accumulation across loop iterations uses memset + start=False.
  - bf16 everywhere on the matmul path (fp32 PSUM), host pre-casts/packs
    so each input is a single contiguous DMA.
  - RoPE head-dim permutation trick: wq/wk columns permuted per head to
    [even|odd] so RoPE is two contiguous 64-partition halves.
  - Causality via a full additive mask tensor indexed by (ktile, qchunk)
    inside the loops (identical on all cores).
"""

import numpy as np
import ml_dtypes
from contextlib import ExitStack

import concourse.bacc as bacc
import concourse.bass as bass
import concourse.tile as tile
import concourse.mybir as mybir
from concourse.bass_utils import run_bass_kernel_spmd

S = 2048
D = 2048
H = 16
KVH = 4
HD = 128
NCORES = 8
F32 = mybir.dt.float32
BF16 = mybir.dt.bfloat16
NPBF16 = ml_dtypes.bfloat16
SCALE = float(1.0 / np.sqrt(HD))
NEG = -1e9

_BUILD_CACHE = {}


def _emit_body(nc, tc, io):
    mm = nc.tensor.matmul
    ds = bass.ds
    with ExitStack() as ctx:
        sb = ctx.enter_context(tc.tile_pool(name="sb", bufs=1))
        dram = ctx.enter_context(tc.tile_pool(name="dram", bufs=1, space="DRAM"))

        ones_sb = sb.tile([128, 1], BF16, tag="ones")
        nc.vector.memset(ones_sb[:], 1.0)
        ident_sb = sb.tile([128, 128], BF16, tag="ident")
        nc.sync.dma_start(out=ident_sb[:], in_=io["ident"][:])
        wqkv_sb = sb.tile([128, 8192], BF16, tag="wqkv")
        nc.sync.dma_start(out=wqkv_sb[:], in_=io["wqkv"][:])
        wo_sb = sb.tile([128, 4096], BF16, tag="wo")
        nc.sync.dma_start(out=wo_sb[:], in_=io["wo2"][:])
        cossin_sb = sb.tile([64, 4096], BF16, tag="cossin")
        nc.sync.dma_start(out=cossin_sb[:], in_=io["cossin"][:])
        mask_sb = sb.tile([128, 32768], BF16, tag="mask")
        nc.sync.dma_start(out=mask_sb[:], in_=io["maskF"][:])

        # persistent activations: qk = [q0 | q1 | kT] each [128, 2048]
        qk_sb = sb.tile([128, 3, 2048], BF16, tag="qk")
        vT_sb = sb.tile([128, 2048], BF16, tag="vT")
        v_sb = sb.tile([128, 2048], BF16, tag="v")
        attnT_sb = sb.tile([128, 4096], BF16, tag="attnT")   # [hd, (h, q)]
        recip_sb = sb.tile([1, 4096], F32, tag="recip")

        xT_r = io["xT"].rearrange("(t p) s -> p t s", p=128)  # [128, 16, 2048]

        # ---------------- phase 1: QKV projections (transposed) --------------
        # loop sc (s-chunks of 512): stream x chunk, stage wqkv[dt], 4 MMs/dt
        with tc.tile_pool(name="p1", bufs=1) as p1, \
             tc.tile_pool(name="p1s", bufs=1) as p1s, \
             tc.tile_pool(name="pp1", bufs=1, space="PSUM") as pp1:
            ps_p = pp1.tile([128, 4, 512], F32, tag="ps_p")  # q0,q1,k,v
            xch = p1.tile([128, 16, 512], BF16, tag="xch")
            wst = p1s.tile([128, 512], BF16, tag="wst")

            xch_f = xch[:].rearrange("p a b -> p (a b)")

            def p1_dt(dt, unroll=1):
                nc.vector.tensor_copy(wst[:], wqkv_sb[:, ds(dt * 512, 512)])
                for j in range(4):
                    mm(ps_p[:, j, :], wst[:, j * 128:(j + 1) * 128],
                       xch_f[:, ds(dt * 512, 512)], start=False, stop=False)

            def p1_body(sc, unroll=1):
                nc.sync.dma_start(out=xch[:], in_=xT_r[:, :, ds(sc * 512, 512)])
                nc.vector.memset(ps_p[:], 0.0)
                with tc.For_i(0, 16, 1) as dt:
                    p1_dt(dt)
                for j in range(3):
                    nc.vector.tensor_copy(qk_sb[:, j, ds(sc * 512, 512)],
                                          ps_p[:, j, :])
                nc.vector.tensor_copy(vT_sb[:, ds(sc * 512, 512)], ps_p[:, 3, :])

            with tc.For_i(0, 4, 1) as sc:
                p1_body(sc)

        # ---------------- phase 1b: vT -> v natural (PE transposes) ----------
        with tc.tile_pool(name="p2s", bufs=1) as p2s, \
             tc.tile_pool(name="pp2", bufs=1, space="PSUM") as pp2:
            tst = p2s.tile([128, 128], BF16, tag="tst")
            ps_t = pp2.tile([128, 128], BF16, tag="ps_t")

            def p2_body(kt, unroll=1):
                nc.vector.tensor_copy(tst[:], vT_sb[:, ds(kt * 128, 128)])
                nc.tensor.transpose(ps_t[:], tst[:], ident_sb[:])
                nc.vector.tensor_copy(v_sb[:, ds(kt * 128, 128)], ps_t[:])

            with tc.For_i(0, 16, 1) as kt:
                p2_body(kt)

        # ---------------- phase 1.5: RoPE on q0, q1, k (in place) ------------
        with tc.tile_pool(name="rp", bufs=1) as rp:
            cosb = cossin_sb[:, 0:2048].unsqueeze(1).broadcast_to((64, 3, 2048))
            sinb = cossin_sb[:, 2048:4096].unsqueeze(1).broadcast_to((64, 3, 2048))
            t0 = qk_sb[0:64, :, :]
            t1c = rp.tile([64, 3, 2048], BF16, tag="t1c")
            nc.sync.dma_start(out=t1c[:], in_=qk_sb[64:128, :, :])
            o1 = rp.tile([64, 3, 2048], BF16, tag="o1")
            tmp = rp.tile([64, 3, 2048], BF16, tag="tmp")
            nc.vector.tensor_mul(o1[:], t0, sinb)
            nc.vector.tensor_mul(tmp[:], t1c[:], cosb)
            nc.vector.tensor_add(o1[:], o1[:], tmp[:])
            nc.vector.tensor_mul(tmp[:], t1c[:], sinb)
            nc.vector.tensor_mul(t1c[:], t0, cosb)
            nc.vector.tensor_sub(qk_sb[0:64, :, :], t1c[:], tmp[:])
            nc.sync.dma_start(out=qk_sb[64:128, :, :], in_=o1[:])

        # ---------------- phase 2: attention (h x qc x kt loops) -------------
        qk_f = qk_sb[:].rearrange("p a b -> p (a b)")        # [128, 6144]
        with tc.tile_pool(name="p3s", bufs=1) as p3s, \
             tc.tile_pool(name="pp3", bufs=1, space="PSUM") as pp3:
            kst = p3s.tile([128, 128], BF16, tag="kst")
            vst = p3s.tile([128, 128], BF16, tag="vst")
            probs = p3s.tile([128, 512], BF16, tag="probs")
            ps_sc = pp3.tile([128, 512], F32, tag="ps_sc")
            ps_pv = pp3.tile([128, 512], F32, tag="ps_pv")
            ps_den = pp3.tile([1, 512], F32, tag="ps_den")

            def kt_body(h, qc, kt, unroll=1):
                nc.vector.tensor_copy(kst[:], qk_f[:, ds(4096 + kt * 128, 128)])
                nc.vector.tensor_copy(vst[:], v_sb[:, ds(kt * 128, 128)])
                mm(ps_sc[:], kst[:], qk_f[:, ds(h * 2048 + qc * 512, 512)],
                   start=True, stop=True)
                nc.vector.tensor_add(
                    ps_sc[:], ps_sc[:],
                    mask_sb[:, ds(kt * 2048 + qc * 512, 512)])
                nc.scalar.activation(probs[:], ps_sc[:],
                                     mybir.ActivationFunctionType.Exp,
                                     scale=SCALE)
                mm(ps_pv[:], vst[:], probs[:], start=False, stop=False)
                mm(ps_den[:], ones_sb[:], probs[:], start=False, stop=False)

            def qc_body(h, qc, unroll=1):
                nc.vector.memset(ps_pv[:], 0.0)
                nc.vector.memset(ps_den[:], 0.0)
                # causal: q-chunk qc only attends k-tiles 0..4*qc+3
                with tc.For_i(0, qc * 4 + 4, 1) as kt:
                    kt_body(h, qc, kt)
                nc.vector.tensor_copy(attnT_sb[:, ds(h * 2048 + qc * 512, 512)],
                                      ps_pv[:])
                nc.vector.reciprocal(recip_sb[:, ds(h * 2048 + qc * 512, 512)],
                                     ps_den[:])

            with tc.For_i(0, 2, 1) as h:
                with tc.For_i(0, 4, 1) as qc:
                    qc_body(h, qc)

        # normalize: DRAM-bounce broadcast of 1/den, one big multiply
        rb = dram.tile([1, 4096], F32, name="rb")
        nc.sync.dma_start(out=rb[:], in_=recip_sb[:])
        bc = sb.tile([128, 4096], F32, tag="bc")
        nc.sync.dma_start(out=bc[:], in_=rb.to_broadcast((128, 4096)))
        nc.vector.tensor_mul(attnT_sb[:], attnT_sb[:], bc[:])

        # ---------------- phase 3: output projection (outT = wo^T attnT) -----
        # outT[n, q] = sum_f wo[f, n] attnT[f, q]; loop over (nh, qc)
        woaccT = dram.tile([128, 16, 2048], BF16, name="woaccT")
        with tc.tile_pool(name="p4s", bufs=1) as p4s, \
             tc.tile_pool(name="pp4", bufs=1, space="PSUM") as pp4:
            wst4 = p4s.tile([128, 2, 1024], BF16, tag="wst4")
            o_sb = p4s.tile([128, 8, 512], BF16, tag="osb")
            ps_wo = pp4.tile([128, 8, 512], F32, tag="ps_wo")

            def p4_body(qc, unroll=1):
                for nh in range(2):
                    # wo packed as [p, nh, f, n1024]
                    nc.vector.tensor_copy(
                        wst4[:].rearrange("p a b -> p (a b)"),
                        wo_sb[:, nh * 2048:(nh + 1) * 2048])
                    for nt in range(8):
                        for f in range(2):
                            mm(ps_wo[:, nt, :],
                               wst4[:, f, nt * 128:(nt + 1) * 128],
                               attnT_sb[:, ds(f * 2048 + qc * 512, 512)],
                               start=f == 0, stop=f == 1)
                    nc.vector.tensor_copy(o_sb[:], ps_wo[:])
                    nc.sync.dma_start(
                        out=woaccT[:, nh * 8:(nh + 1) * 8, ds(qc * 512, 512)],
                        in_=o_sb[:])

            with tc.For_i(0, 4, 1) as qc:
                p4_body(qc)

        # ---------------- ReduceScatter over cores (bf16) --------------------
        rs_out = dram.tile([16, 16, 2048], BF16, name="rsout")
        nc.gpsimd.collective_compute(
            "ReduceScatter", mybir.AluOpType.add,
            replica_groups=[list(range(NCORES))],
            ins=[woaccT.opt()], outs=[rs_out.opt()])
        nc.sync.dma_start(out=io["out"][:], in_=rs_out[:])


def build(repeat=1, num_devices=NCORES):
    key = (repeat, num_devices)
    if key in _BUILD_CACHE:
        return _BUILD_CACHE[key]
    nc = bacc.Bacc("TRN2", target_bir_lowering=False, debug=False,
                   num_devices=num_devices)
    io = {
        "xT": nc.dram_tensor("xT", [D, S], BF16, kind="ExternalInput").ap(),
        "wqkv": nc.dram_tensor("wqkv", [128, 8192], BF16,
                               kind="ExternalInput").ap(),
        "wo2": nc.dram_tensor("wo2", [128, 4096], BF16,
                              kind="ExternalInput").ap(),
        "cossin": nc.dram_tensor("cossin", [64, 4096], BF16,
                                 kind="ExternalInput").ap(),
        "maskF": nc.dram_tensor("maskF", [128, 32768], BF16,
                                kind="ExternalInput").ap(),
        "ident": nc.dram_tensor("ident", [128, 128], BF16,
                                kind="ExternalInput").ap(),
        "out": nc.dram_tensor("out", [16, 16, 2048], BF16,
                              kind="ExternalOutput").ap(),
    }
    with tile.TileContext(nc) as tc:
        for _ in range(repeat):
            _emit_body(nc, tc, io)
    nc.compile()
    _BUILD_CACHE[key] = nc
    return nc


def prepare_in_maps(x, wq, wk, wv, wo, freqs_cos, freqs_sin):
    bf = lambda a: np.ascontiguousarray(a).astype(NPBF16)
    x2d = np.asarray(x, dtype=np.float32).reshape(S, D)
    xT = bf(x2d.T)
    cosT = np.asarray(freqs_cos, np.float32).T                # [64, S]
    sinT = np.asarray(freqs_sin, np.float32).T
    cossin = bf(np.concatenate([cosT, sinT], axis=1))         # [64, 4096]

    perm = np.concatenate([np.arange(0, HD, 2), np.arange(1, HD, 2)])
    wq = np.asarray(wq, np.float32)
    wk = np.asarray(wk, np.float32)
    wv = np.asarray(wv, np.float32)
    wo = np.asarray(wo, np.float32)
    wqP = wq.reshape(D, H, HD)[:, :, perm]                    # [D, 16, 128]
    wkP = wk.reshape(D, KVH, HD)[:, :, perm]                  # [D, 4, 128]
    wv4 = wv.reshape(D, KVH, HD)

    # full causal mask in scoresT layout: [kl, kt, qc, ql]
    kl = np.arange(128)[:, None, None, None]
    ktv = np.arange(16)[None, :, None, None]
    qcv = np.arange(4)[None, None, :, None]
    qlv = np.arange(512)[None, None, None, :]
    maskF = np.where(128 * ktv + kl <= 512 * qcv + qlv, 0.0, NEG)
    maskF = bf(maskF.reshape(128, 32768))

    ident = bf(np.eye(128, dtype=np.float32))

    in_maps = []
    for c in range(NCORES):
        g = c // 2
        # wqkv packed [p, dt*512 + (q0|q1|k|v)*128 + col]
        wqkv = np.stack([wqP[:, 2 * c, :], wqP[:, 2 * c + 1, :],
                         wkP[:, g, :], wv4[:, g, :]], axis=1)  # [D, 4, 128]
        wqkv = bf(wqkv.reshape(16, 128, 512).transpose(1, 0, 2)
                  .reshape(128, 8192))
        # wo rows for this core's heads, packed [p, nh, f, n1024]
        woc = wo[256 * c:256 * c + 256, :]                    # [256, 2048]
        wo2 = bf(woc.reshape(2, 128, 2, 1024).transpose(1, 2, 0, 3)
                 .reshape(128, 4096))
        in_maps.append({
            "xT": xT,
            "wqkv": wqkv,
            "wo2": wo2,
            "cossin": cossin,
            "maskF": maskF,
            "ident": ident,
        })
    return in_maps


def assemble_output(results):
    # outT[nt*128 + 16*c + pp, q] = results[c]["out"][pp, nt, q]
    outT = np.empty((2048, 2048), np.float32)
    for c in range(NCORES):
        o = np.asarray(results[c]["out"], np.float32)         # [16, 16, 2048]
        for nt in range(16):
            outT[nt * 128 + 16 * c: nt * 128 + 16 * c + 16, :] = o[:, nt, :]
    return np.ascontiguousarray(outT.T).reshape(1, S, D)


def kernel(x, wq, wk, wv, wo, freqs_cos, freqs_sin, mask):
    nc = build()
    in_maps = prepare_in_maps(x, wq, wk, wv, wo, freqs_cos, freqs_sin)
    res = run_bass_kernel_spmd(nc, in_maps, core_ids=list(range(NCORES)))
    return assemble_output(res.results).astype(np.float32)
